# Initial kernel scaffold
#
"""Trainium2 Bass kernel for nn_Attention (Luong-style attention block).

Computes, per batch b:
    h   = input @ W_in^T + b_in                  [T, D]
    score = h @ memory_bank^T                    [T, S]
    align = softmax(score, axis=-1)              [T, S]
    context = align @ memory_bank                [T, D]
    attn_h = tanh(concat(context, input) @ W_out^T + b_out)   [T, D]
Returns (attn_h, align).

Sharding: data-parallel over batch across 8 NeuronCores (B=16 -> 2 per core).

Precision strategy: the score logits have std ~59 and near-ties in the softmax,
so the h/score matmuls run as 3-term split-bf16 (hi*hi + hi*lo + lo*hi with
fp32 PSUM accumulation) giving ~1e-3 absolute logit error; context/output
matmuls are plain bf16 (errors pass through softmax/tanh attenuated).

All matmuls are laid out feature-major (H^T = [e, t], mem^T = [e, s]) so each
product feeds the next without transposition; only align ([t, s] -> [s, t])
needs PE transposes, done in 128x128 blocks packed 4-per-PSUM-bank.
"""

import numpy as np

N_CORES = 8
PROFILE = False  # set True (e.g. from test.py) to NTFF-profile and print HW time

_COMPILED = {}
LAST_EXEC_NS = None


def _install_ntff_shim():
    """Make run_bass_kernel_spmd(trace=True) work: register the ctypes NTFF
    profile hook under the antenv.axon_hooks name concourse looks up."""
    import sys
    import types

    if "antenv.axon_hooks" in sys.modules:
        return
    try:
        from trn_agent_boot.trn_boot import _ntff_profile_via_ctypes

        hook = _ntff_profile_via_ctypes("/opt/axon/libaxon_pjrt.so")
    except Exception:
        hook = None
    mod = types.ModuleType("antenv.axon_hooks")
    mod.get_axon_ntff_profile_hook = lambda: hook
    mod.set_axon_ntff_profile_hook = lambda h: None
    sys.modules["antenv.axon_hooks"] = mod


def _build(nb, T, S, D):
    """Build + compile the per-core program: nb batches of [T, D] x [S, D]."""
    import concourse.bass as bass  # noqa: F401
    import concourse.tile as tile
    from concourse import bacc, mybir
    from concourse.masks import make_identity

    dt = mybir.dt
    bf = dt.bfloat16
    f32 = dt.float32

    assert D % 128 == 0 and T % 256 == 0 and S % 1024 == 0
    TBLK = 256  # token chunk
    KD = D // 128  # d (input-feature) tiles
    KE = D // 128  # e (hidden-feature) tiles
    KS = S // 128  # s (memory row) tiles
    NCH = T // TBLK  # chunks per batch
    NTT = TBLK // 128  # t-tiles per chunk
    SH = S // 2  # score PSUM half
    NSB = SH // 512  # 512-wide s-blocks per half
    NEH = D // 512  # 512-wide e-halves for the output matmul

    nc = bacc.Bacc("TRN2", target_bir_lowering=False, debug=False)

    def din(name, shape, dtype=bf):
        return nc.dram_tensor(name, shape, dtype, kind="ExternalInput").ap()

    xt_hi = din("xt_hi", [nb, D, T])
    xt_lo = din("xt_lo", [nb, D, T])
    mt_hi = din("mt_hi", [nb, D, S])
    mt_lo = din("mt_lo", [nb, D, S])
    mem_n = din("mem_n", [nb, S, D])
    w_in_hi = din("w_in_hi", [D, D])
    w_in_lo = din("w_in_lo", [D, D])
    w_out_t = din("w_out_t", [2 * D, D])
    b_in_rows = din("b_in_rows", [2, D])
    b_out_rows = din("b_out_rows", [2, D])
    attn_out = nc.dram_tensor("attn_out", [nb, T, D], f32, kind="ExternalOutput").ap()
    align_out = nc.dram_tensor("align_out", [nb, T, S], f32, kind="ExternalOutput").ap()

    with tile.TileContext(nc) as tc:
        with (
            tc.tile_pool(name="consts", bufs=1) as consts,
            tc.tile_pool(name="weights", bufs=1) as weights,
            tc.tile_pool(name="mt", bufs=1) as mtp,
            tc.tile_pool(name="xt", bufs=2) as xtp,
            tc.tile_pool(name="hts", bufs=1) as htp,
            tc.tile_pool(name="exps", bufs=2) as expp,
            tc.tile_pool(name="stats", bufs=4) as statp,
            tc.tile_pool(name="aligns", bufs=1) as alip,
            tc.tile_pool(name="ctxs", bufs=1) as ctxp,
            tc.tile_pool(name="mcols", bufs=2) as mcolp,
            tc.tile_pool(name="attns", bufs=2) as attp,
            tc.tile_pool(name="h_ps", bufs=1, space="PSUM") as h_ps,
            tc.tile_pool(name="sc_a", bufs=1, space="PSUM") as sc_a,
            tc.tile_pool(name="sc_b", bufs=1, space="PSUM") as sc_b,
            tc.tile_pool(name="tr_ps", bufs=1, space="PSUM") as tr_psp,
            tc.tile_pool(name="ctx_ps", bufs=1, space="PSUM") as ctx_psp,
            tc.tile_pool(name="at_ps", bufs=1, space="PSUM") as at_psp,
        ):
            ident = consts.tile([128, 128], bf)
            make_identity(nc, ident)
            ones_tb = consts.tile([2, TBLK], bf)
            nc.vector.memset(ones_tb, 1.0)
            ones_tt = consts.tile([2, 128], bf)
            nc.vector.memset(ones_tt, 1.0)
            sb_b_in = consts.tile([2, D], bf)
            nc.sync.dma_start(out=sb_b_in, in_=b_in_rows)
            sb_b_out = consts.tile([2, D], bf)
            nc.sync.dma_start(out=sb_b_out, in_=b_out_rows)
            sb_w_in_hi = weights.tile([128, KD, D], bf, name="w_in_hi")
            nc.sync.dma_start(
                out=sb_w_in_hi, in_=w_in_hi.rearrange("(k p) e -> p k e", p=128)
            )
            sb_w_in_lo = weights.tile([128, KD, D], bf, name="w_in_lo")
            nc.sync.dma_start(
                out=sb_w_in_lo, in_=w_in_lo.rearrange("(k p) e -> p k e", p=128)
            )
            sb_w_out = weights.tile([128, 2 * KD, D], bf, name="w_out")
            nc.sync.dma_start(
                out=sb_w_out, in_=w_out_t.rearrange("(k p) e -> p k e", p=128)
            )

            for b in range(nb):
                sb_mt_hi = mtp.tile([128, KE, S], bf, name="mt_hi")
                nc.sync.dma_start(
                    out=sb_mt_hi, in_=mt_hi[b].rearrange("(k p) s -> p k s", p=128)
                )
                sb_mt_lo = mtp.tile([128, KE, S], bf, name="mt_lo")
                nc.sync.dma_start(
                    out=sb_mt_lo, in_=mt_lo[b].rearrange("(k p) s -> p k s", p=128)
                )

                for c in range(NCH):
                    tsl = slice(c * TBLK, (c + 1) * TBLK)
                    sb_xt_hi = xtp.tile([128, KD, TBLK], bf, name="xt_hi")
                    nc.sync.dma_start(
                        out=sb_xt_hi,
                        in_=xt_hi[b, :, tsl].rearrange("(k p) t -> p k t", p=128),
                    )
                    sb_xt_lo = xtp.tile([128, KD, TBLK], bf, name="xt_lo")
                    nc.sync.dma_start(
                        out=sb_xt_lo,
                        in_=xt_lo[b, :, tsl].rearrange("(k p) t -> p k t", p=128),
                    )

                    # ---- phase A: H^T (hi/lo bf16) for this chunk ----
                    sb_h_hi = htp.tile([128, KE, TBLK], bf, name="h_hi")
                    sb_h_lo = htp.tile([128, KE, TBLK], bf, name="h_lo")
                    for e in range(KE):
                        esl = slice(e * 128, (e + 1) * 128)
                        hps = h_ps.tile([128, TBLK], f32)
                        for k in range(KD):
                            nc.tensor.matmul(
                                hps,
                                sb_w_in_hi[:, k, esl],
                                sb_xt_hi[:, k, :],
                                start=(k == 0),
                                stop=False,
                            )
                            nc.tensor.matmul(
                                hps, sb_w_in_hi[:, k, esl], sb_xt_lo[:, k, :],
                                start=False, stop=False,
                            )
                            nc.tensor.matmul(
                                hps, sb_w_in_lo[:, k, esl], sb_xt_hi[:, k, :],
                                start=False, stop=False,
                            )
                        nc.tensor.matmul(
                            hps, sb_b_in[:, esl], ones_tb, start=False, stop=True
                        )
                        nc.scalar.copy(sb_h_hi[:, e, :], hps)
                        nc.vector.tensor_sub(sb_h_lo[:, e, :], hps, sb_h_hi[:, e, :])

                    sb_alignT = alip.tile([128, KS, TBLK], bf, name="alignT")

                    # ---- phase B: score + softmax + transpose, per t-tile ----
                    for tt in range(NTT):
                        t0 = tt * 128
                        tt_sl = slice(t0, t0 + 128)
                        gt0 = c * TBLK + t0
                        pa = sc_a.tile([128, SH], f32)
                        pb = sc_b.tile([128, SH], f32)
                        for ps_half, s_base in ((pa, 0), (pb, SH)):
                            for k in range(KE):
                                h_hi_sl = sb_h_hi[:, k, tt_sl]
                                h_lo_sl = sb_h_lo[:, k, tt_sl]
                                for term, (lhs, rhs_t) in enumerate(
                                    ((h_hi_sl, sb_mt_hi), (h_hi_sl, sb_mt_lo),
                                     (h_lo_sl, sb_mt_hi))
                                ):
                                    for sb in range(NSB):
                                        ssl = slice(
                                            s_base + sb * 512, s_base + (sb + 1) * 512
                                        )
                                        nc.tensor.matmul(
                                            ps_half[:, sb * 512 : (sb + 1) * 512],
                                            lhs,
                                            rhs_t[:, k, ssl],
                                            start=(k == 0 and term == 0),
                                            stop=(k == KE - 1 and term == 2),
                                            skip_group_check=True,
                                        )
                        nmax_a = statp.tile([128, 1], f32, name="nmax_a")
                        nmax_b = statp.tile([128, 1], f32, name="nmax_b")
                        nmax = statp.tile([128, 1], f32, name="nmax")
                        nc.vector.reduce_max(
                            nmax_a, pa, axis=mybir.AxisListType.X, negate=True
                        )
                        nc.vector.reduce_max(
                            nmax_b, pb, axis=mybir.AxisListType.X, negate=True
                        )
                        nc.vector.tensor_tensor(
                            out=nmax, in0=nmax_a, in1=nmax_b, op=mybir.AluOpType.min
                        )
                        exp_t = expp.tile([128, S], bf, name="exp_t")
                        sums_a = statp.tile([128, 1], f32, name="sums_a")
                        sums_b = statp.tile([128, 1], f32, name="sums_b")
                        nc.scalar.activation(
                            exp_t[:, :SH], pa, mybir.ActivationFunctionType.Exp,
                            bias=nmax, accum_out=sums_a,
                        )
                        nc.scalar.activation(
                            exp_t[:, SH:], pb, mybir.ActivationFunctionType.Exp,
                            bias=nmax, accum_out=sums_b,
                        )
                        rsum = statp.tile([128, 1], f32, name="rsum")
                        nc.vector.tensor_add(rsum, sums_a, sums_b)
                        nc.vector.reciprocal(rsum, rsum)
                        nc.vector.tensor_scalar_mul(exp_t, exp_t, rsum)
                        nc.gpsimd.dma_start(
                            out=align_out[b, gt0 : gt0 + 128, :], in_=exp_t
                        )
                        for g in range(KS // 4):
                            trp = tr_psp.tile([128, 512], f32)
                            for q in range(4):
                                sk = g * 4 + q
                                nc.tensor.matmul(
                                    trp[:, q * 128 : (q + 1) * 128],
                                    exp_t[:, sk * 128 : (sk + 1) * 128],
                                    ident,
                                    is_transpose=True,
                                    start=(q == 0),
                                    stop=(q == 3),
                                    skip_group_check=True,
                                )
                            nc.scalar.copy(
                                sb_alignT[:, g * 4 : (g + 1) * 4, tt_sl],
                                trp.rearrange("p (q t) -> p q t", q=4),
                            )

                    # ---- phase C: context^T ----
                    sb_ctxT = ctxp.tile([128, KD, TBLK], bf, name="ctxT")
                    for dj in range(KD):
                        dsl = slice(dj * 128, (dj + 1) * 128)
                        mcol = mcolp.tile([128, KS, 128], bf, name="mcol")
                        nc.sync.dma_start(
                            out=mcol,
                            in_=mem_n[b, :, dsl].rearrange("(sk p) d -> p sk d", p=128),
                        )
                        cps = ctx_psp.tile([128, TBLK], f32)
                        for sk in range(KS):
                            nc.tensor.matmul(
                                cps,
                                mcol[:, sk, :],
                                sb_alignT[:, sk, :],
                                start=(sk == 0),
                                stop=(sk == KS - 1),
                            )
                        nc.scalar.copy(sb_ctxT[:, dj, :], cps)

                    # ---- phase D: attn_h = tanh(concat @ W_out^T + b_out) ----
                    for tt in range(NTT):
                        t0 = tt * 128
                        tt_sl = slice(t0, t0 + 128)
                        gt0 = c * TBLK + t0
                        asb = attp.tile([128, D], f32, name="attn_sb")
                        for eh in range(NEH):
                            esl = slice(eh * 512, (eh + 1) * 512)
                            aps = at_psp.tile([128, 512], f32)
                            for dj in range(KD):
                                nc.tensor.matmul(
                                    aps,
                                    sb_ctxT[:, dj, tt_sl],
                                    sb_w_out[:, dj, esl],
                                    start=(dj == 0),
                                    stop=False,
                                )
                            for dk in range(KD):
                                nc.tensor.matmul(
                                    aps,
                                    sb_xt_hi[:, dk, tt_sl],
                                    sb_w_out[:, KD + dk, esl],
                                    start=False,
                                    stop=False,
                                )
                            nc.tensor.matmul(
                                aps, ones_tt, sb_b_out[:, esl], start=False, stop=True
                            )
                            nc.scalar.activation(
                                asb[:, esl], aps, mybir.ActivationFunctionType.Tanh
                            )
                        nc.sync.dma_start(
                            out=attn_out[b, gt0 : gt0 + 128, :], in_=asb
                        )

    nc.compile()
    return nc


def _split_bf16(x):
    import ml_dtypes

    bf16 = ml_dtypes.bfloat16
    hi = x.astype(bf16)
    lo = (x - hi.astype(np.float32)).astype(bf16)
    return hi, lo


def kernel(input, memory_bank, W_in, b_in, W_out, b_out):
    import ml_dtypes
    from concourse.bass_utils import run_bass_kernel_spmd

    global LAST_EXEC_NS
    bf16 = ml_dtypes.bfloat16
    input = np.asarray(input, dtype=np.float32)
    memory_bank = np.asarray(memory_bank, dtype=np.float32)
    W_in = np.asarray(W_in, dtype=np.float32)
    b_in = np.asarray(b_in, dtype=np.float32)
    W_out = np.asarray(W_out, dtype=np.float32)
    b_out = np.asarray(b_out, dtype=np.float32)

    B, T, D = input.shape
    S = memory_bank.shape[1]
    assert B % N_CORES == 0
    nb = B // N_CORES

    key = (nb, T, S, D)
    if key not in _COMPILED:
        _COMPILED[key] = _build(*key)
    nc = _COMPILED[key]

    xt = np.ascontiguousarray(input.transpose(0, 2, 1))  # [B, D, T]
    xt_hi, xt_lo = _split_bf16(xt)
    mt = np.ascontiguousarray(memory_bank.transpose(0, 2, 1))  # [B, D, S]
    mt_hi, mt_lo = _split_bf16(mt)
    mem_n = memory_bank.astype(bf16)
    w_in_t = np.ascontiguousarray(W_in.T)  # [d, e]
    w_in_hi, w_in_lo = _split_bf16(w_in_t)
    w_out_t = np.ascontiguousarray(W_out.T).astype(bf16)  # [f, e]
    b_in_rows = np.stack(_split_bf16(b_in))  # [2, D]
    b_out_rows = np.stack(_split_bf16(b_out))

    in_maps = []
    for c in range(N_CORES):
        bsl = slice(c * nb, (c + 1) * nb)
        in_maps.append(
            {
                "xt_hi": xt_hi[bsl],
                "xt_lo": xt_lo[bsl],
                "mt_hi": mt_hi[bsl],
                "mt_lo": mt_lo[bsl],
                "mem_n": mem_n[bsl],
                "w_in_hi": w_in_hi,
                "w_in_lo": w_in_lo,
                "w_out_t": w_out_t,
                "b_in_rows": b_in_rows,
                "b_out_rows": b_out_rows,
            }
        )

    if PROFILE:
        _install_ntff_shim()
    res = run_bass_kernel_spmd(nc, in_maps, list(range(N_CORES)), trace=PROFILE)
    if PROFILE:
        LAST_EXEC_NS = res.exec_time_ns
        print(f"HW exec time: {res.exec_time_ns} ns")

    attn_h = np.concatenate([res.results[c]["attn_out"] for c in range(N_CORES)])
    align = np.concatenate([res.results[c]["align_out"] for c in range(N_CORES)])
    return attn_h, align


# revision 4
# speedup vs baseline: 7.4963x; 7.4963x over previous
"""Trainium2 Bass kernel for nn_Attention (Luong-style attention block).

Per batch b:
    h     = input @ W_in^T + b_in                 [T, D]
    score = h @ memory_bank^T                     [T, S]
    align = softmax(score, axis=-1)               [T, S]
    context = align @ memory_bank                 [T, D]
    attn_h = tanh(concat(context, input) @ W_out^T + b_out)   [T, D]
Returns (attn_h, align).

Sharding: data-parallel over batch across 8 NeuronCores (B=16 -> 2 per core).

Precision: score logits have std ~59 with softmax near-ties, so h/score
matmuls run as 3-term split-bf16 (hi*hi + hi*lo + lo*hi, fp32 PSUM) for
~1e-3 logit error. The context and output matmuls run in float32r
(fp32-stored, ~13-bit-mantissa matmul) — full bf16 throughput at N>=256
with ~1e-4 relative error.

Layouts are feature-major (H^T=[e,t], mem^T=[e,s]) so products chain without
transposes; only align ([t,s]->[s,t]) is PE-transposed in 128x128 blocks,
4 packed per PSUM bank. Phase A (h) runs batch-level with H^T spilled to a
DRAM scratch so the W_in and W_out residencies never coexist in SBUF.
"""

from contextlib import ExitStack

import numpy as np

N_CORES = 8
PROFILE = False  # set True (e.g. from test.py) to NTFF-profile and print HW time

_COMPILED = {}
LAST_EXEC_NS = None
LAST_RESULTS = None


def _install_ntff_shim():
    import sys
    import types

    if "antenv.axon_hooks" in sys.modules:
        return
    try:
        from trn_agent_boot.trn_boot import _ntff_profile_via_ctypes

        hook = _ntff_profile_via_ctypes("/opt/axon/libaxon_pjrt.so")
    except Exception:
        hook = None
    mod = types.ModuleType("antenv.axon_hooks")
    mod.get_axon_ntff_profile_hook = lambda: hook
    mod.set_axon_ntff_profile_hook = lambda h: None
    sys.modules["antenv.axon_hooks"] = mod


class _Prog:
    """Program-under-construction state: nc, pools, tiles, dims."""


def _phase_a(p, b):
    """H^T hi/lo (split-bf16 h matmul + b_in) for the whole batch -> DRAM."""
    nc = p.nc
    for ca in range(p.T // p.TA):
        tsl = slice(ca * p.TA, (ca + 1) * p.TA)
        xa_hi = p.xtA.tile([128, p.KD, p.TA], p.bf, name="xa_hi")
        nc.sync.dma_start(
            out=xa_hi, in_=p.xt_hi[b, :, tsl].rearrange("(k p) t -> p k t", p=128))
        xa_lo = p.xtA.tile([128, p.KD, p.TA], p.bf, name="xa_lo")
        nc.sync.dma_start(
            out=xa_lo, in_=p.xt_lo[b, :, tsl].rearrange("(k p) t -> p k t", p=128))
        ha_hi = p.hA.tile([128, p.KE, p.TA], p.bf, name="ha_hi")
        ha_lo = p.hA.tile([128, p.KE, p.TA], p.bf, name="ha_lo")
        for e in range(p.KE):
            esl = slice(e * 128, (e + 1) * 128)
            hps = p.h_ps.tile([128, p.TA], p.f32)
            for k in range(p.KD):
                nc.tensor.matmul(hps, p.sb_w_in_hi[:, k, esl], xa_hi[:, k, :],
                                 start=(k == 0), stop=False)
                nc.tensor.matmul(hps, p.sb_w_in_hi[:, k, esl], xa_lo[:, k, :],
                                 start=False, stop=False)
                nc.tensor.matmul(hps, p.sb_w_in_lo[:, k, esl], xa_hi[:, k, :],
                                 start=False, stop=False)
            nc.tensor.matmul(hps, p.sb_b_in[:, esl], p.ones_tb,
                             start=False, stop=True)
            nc.scalar.copy(ha_hi[:, e, :], hps)
            nc.vector.tensor_sub(ha_lo[:, e, :], hps, ha_hi[:, e, :])
        nc.sync.dma_start(
            out=p.h_scr_hi[b, :, tsl].rearrange("(k p) t -> p k t", p=128),
            in_=ha_hi)
        nc.sync.dma_start(
            out=p.h_scr_lo[b, :, tsl].rearrange("(k p) t -> p k t", p=128),
            in_=ha_lo)


def _phase_b(p, b, c, tt, h_hi, h_lo, alignT):
    """score (split-bf16) -> softmax (f32r) -> align out -> align^T."""
    nc = p.nc
    mybir = p.mybir
    t0 = tt * 128
    tt_sl = slice(t0, t0 + 128)
    gt0 = c * p.TBLK + t0
    pa = p.sc_a.tile([128, p.SH], p.f32)
    pb = p.sc_b.tile([128, p.SH], p.f32)
    for ps_half, s_base in ((pa, 0), (pb, p.SH)):
        for k in range(p.KE):
            h_hi_sl = h_hi[:, k, tt_sl]
            h_lo_sl = h_lo[:, k, tt_sl]
            terms = ((h_hi_sl, p.sb_mt_hi), (h_hi_sl, p.sb_mt_lo),
                     (h_lo_sl, p.sb_mt_hi))
            for term, (lhs, rhs_t) in enumerate(terms):
                for sb in range(p.NSB):
                    ssl = slice(s_base + sb * 512, s_base + (sb + 1) * 512)
                    nc.tensor.matmul(
                        ps_half[:, sb * 512:(sb + 1) * 512], lhs, rhs_t[:, k, ssl],
                        start=(k == 0 and term == 0),
                        stop=(k == p.KE - 1 and term == 2),
                        skip_group_check=True,
                    )
    nmax_a = p.statp.tile([128, 1], p.f32, name="nmax_a")
    nmax_b = p.statp.tile([128, 1], p.f32, name="nmax_b")
    nmax = p.statp.tile([128, 1], p.f32, name="nmax")
    nc.vector.reduce_max(nmax_a, pa, axis=mybir.AxisListType.X, negate=True)
    nc.vector.reduce_max(nmax_b, pb, axis=mybir.AxisListType.X, negate=True)
    nc.vector.tensor_tensor(out=nmax, in0=nmax_a, in1=nmax_b,
                            op=mybir.AluOpType.min)
    exp_t = p.expp.tile([128, p.S], p.f32r, name="exp_t")
    sums_a = p.statp.tile([128, 1], p.f32, name="sums_a")
    sums_b = p.statp.tile([128, 1], p.f32, name="sums_b")
    nc.scalar.activation(exp_t[:, :p.SH], pa, mybir.ActivationFunctionType.Exp,
                         bias=nmax, accum_out=sums_a)
    nc.scalar.activation(exp_t[:, p.SH:], pb, mybir.ActivationFunctionType.Exp,
                         bias=nmax, accum_out=sums_b)
    rsum = p.statp.tile([128, 1], p.f32, name="rsum")
    nc.vector.tensor_add(rsum, sums_a, sums_b)
    nc.vector.reciprocal(rsum, rsum)
    nc.vector.tensor_scalar_mul(exp_t, exp_t, rsum)
    nc.sync.dma_start(out=p.align_out[b, gt0:gt0 + 128, :],
                      in_=exp_t.bitcast(p.f32))
    for g in range(p.KS // 4):
        trp = p.tr_psp.tile([128, 512], p.f32r)
        for q in range(4):
            sk = g * 4 + q
            nc.tensor.matmul(
                trp[:, q * 128:(q + 1) * 128], exp_t[:, sk * 128:(sk + 1) * 128],
                p.ident, is_transpose=True, start=(q == 0), stop=(q == 3),
                skip_group_check=True,
            )
        nc.scalar.copy(alignT[:, g * 4:(g + 1) * 4, tt_sl],
                       trp.rearrange("p (q t) -> p q t", q=4))


def _phase_c(p, b, alignT, ctxT):
    """context^T[d, t] = sum_s mem[s, d] * align^T[s, t]  (f32r)."""
    nc = p.nc
    for dj in range(p.KD):
        dsl = slice(dj * 128, (dj + 1) * 128)
        mcol = p.mcolp.tile([128, p.KS, 128], p.f32r, name="mcol")
        nc.sync.dma_start(
            out=mcol,
            in_=p.mem_n[b, :, dsl].rearrange("(sk p) d -> p sk d", p=128))
        cps = p.ctx_psp.tile([128, p.TBLK], p.f32)
        for sk in range(p.KS):
            nc.tensor.matmul(cps, mcol[:, sk, :], alignT[:, sk, :],
                             start=(sk == 0), stop=(sk == p.KS - 1))
        nc.scalar.copy(ctxT[:, dj, :], cps)


def _phase_d(p, b, c, tt, ctxT, xt_r):
    """attn_h = tanh([context; input] @ W_out^T + b_out)  (f32r matmul)."""
    nc = p.nc
    mybir = p.mybir
    t0 = tt * 128
    tt_sl = slice(t0, t0 + 128)
    gt0 = c * p.TBLK + t0
    for eh in range(p.NEH):
        esl = slice(eh * 512, (eh + 1) * 512)
        aps = p.at_psp.tile([128, 512], p.f32)
        for dj in range(p.KD):
            nc.tensor.matmul(aps, ctxT[:, dj, tt_sl], p.sb_w_out[:, dj, esl],
                             start=(dj == 0), stop=False)
        for dk in range(p.KD):
            nc.tensor.matmul(aps, xt_r[:, dk, tt_sl],
                             p.sb_w_out[:, p.KD + dk, esl], start=False, stop=False)
        nc.tensor.matmul(aps, p.ones_r, p.sb_b_out[:, esl], start=False, stop=True)
        asb = p.attp.tile([128, 512], p.f32, name="attn_st")
        nc.scalar.activation(asb, aps, mybir.ActivationFunctionType.Tanh)
        nc.sync.dma_start(out=p.attn_out[b, gt0:gt0 + 128, esl], in_=asb)


def _build_body(p, tc):
    nc = p.nc
    # constants (global)
    p.ident = p.consts.tile([128, 128], p.f32r)
    nc.sync.dma_start(out=p.ident, in_=p.ident_d)
    p.ones_tb = p.consts.tile([2, p.TA], p.bf)
    nc.vector.memset(p.ones_tb, 1.0)
    p.ones_r = p.consts.tile([1, 128], p.f32r)
    nc.sync.dma_start(out=p.ones_r, in_=p.ones_d)
    p.sb_b_in = p.consts.tile([2, p.D], p.bf)
    nc.sync.dma_start(out=p.sb_b_in, in_=p.b_in_rows)
    p.sb_b_out = p.consts.tile([1, p.D], p.f32r)
    nc.sync.dma_start(out=p.sb_b_out, in_=p.b_out_row)

    for b in range(p.nb):
        p.sb_mt_hi = p.mtp.tile([128, p.KE, p.S], p.bf, name="mt_hi")
        nc.sync.dma_start(out=p.sb_mt_hi,
                          in_=p.mt_hi[b].rearrange("(k p) s -> p k s", p=128))
        p.sb_mt_lo = p.mtp.tile([128, p.KE, p.S], p.bf, name="mt_lo")
        nc.sync.dma_start(out=p.sb_mt_lo,
                          in_=p.mt_lo[b].rearrange("(k p) s -> p k s", p=128))

        # ---- phase A (own pool scope; W_in only lives here) ----
        with ExitStack() as actx:
            p.w_inp = actx.enter_context(tc.tile_pool(name="w_inp", bufs=1))
            p.xtA = actx.enter_context(tc.tile_pool(name="xtA", bufs=2))
            p.hA = actx.enter_context(tc.tile_pool(name="hA", bufs=2))
            p.h_ps = actx.enter_context(
                tc.tile_pool(name="h_ps", bufs=2, space="PSUM"))
            p.sb_w_in_hi = p.w_inp.tile([128, p.KD, p.D], p.bf, name="w_in_hi")
            nc.sync.dma_start(out=p.sb_w_in_hi,
                              in_=p.w_in_hi.rearrange("(k p) e -> p k e", p=128))
            p.sb_w_in_lo = p.w_inp.tile([128, p.KD, p.D], p.bf, name="w_in_lo")
            nc.sync.dma_start(out=p.sb_w_in_lo,
                              in_=p.w_in_lo.rearrange("(k p) e -> p k e", p=128))
            _phase_a(p, b)

        # ---- phases B/C/D (W_out / f32r pools live here) ----
        with ExitStack() as bctx:
            pool = lambda name, bufs, **kw: bctx.enter_context(
                tc.tile_pool(name=name, bufs=bufs, **kw))
            p.w_outp = pool("w_outp", 1)
            p.hBp = pool("hB", 1)
            p.xtBp = pool("xtB", 2)
            p.expp = pool("exps", 1)
            p.alip = pool("aligns", 1)
            p.ctxp = pool("ctxs", 1)
            p.mcolp = pool("mcols", 2)
            p.attp = pool("attns", 2)
            p.sc_a = pool("sc_a", 1, space="PSUM")
            p.sc_b = pool("sc_b", 1, space="PSUM")
            p.tr_psp = pool("tr_ps", 1, space="PSUM")
            p.ctx_psp = pool("ctx_ps", 1, space="PSUM")
            p.at_psp = pool("at_ps", 1, space="PSUM")

            p.sb_w_out = p.w_outp.tile([128, 2 * p.KD, p.D], p.f32r, name="w_out")
            nc.sync.dma_start(out=p.sb_w_out,
                              in_=p.w_out_t.rearrange("(k p) e -> p k e", p=128))
            for c in range(p.NCH):
                tsl = slice(c * p.TBLK, (c + 1) * p.TBLK)
                h_hi = p.hBp.tile([128, p.KE, p.TBLK], p.bf, name="h_hi")
                nc.sync.dma_start(
                    out=h_hi,
                    in_=p.h_scr_hi[b, :, tsl].rearrange("(k p) t -> p k t", p=128))
                h_lo = p.hBp.tile([128, p.KE, p.TBLK], p.bf, name="h_lo")
                nc.sync.dma_start(
                    out=h_lo,
                    in_=p.h_scr_lo[b, :, tsl].rearrange("(k p) t -> p k t", p=128))
                xt_r = p.xtBp.tile([128, p.KD, p.TBLK], p.f32r, name="xt_r")
                nc.sync.dma_start(
                    out=xt_r,
                    in_=p.xt_f32[b, :, tsl].rearrange("(k p) t -> p k t", p=128))
                alignT = p.alip.tile([128, p.KS, p.TBLK], p.f32r, name="alignT")
                for tt in range(p.NTT):
                    _phase_b(p, b, c, tt, h_hi, h_lo, alignT)
                ctxT = p.ctxp.tile([128, p.KD, p.TBLK], p.f32r, name="ctxT")
                _phase_c(p, b, alignT, ctxT)
                for tt in range(p.NTT):
                    _phase_d(p, b, c, tt, ctxT, xt_r)


def _build(nb, T, S, D):
    """Build + compile the per-core program: nb batches of [T, D] x [S, D]."""
    import concourse.bass as bass  # noqa: F401
    import concourse.tile as tile
    from concourse import bacc, mybir

    p = _Prog()
    p.mybir = mybir
    dt = mybir.dt
    p.bf = dt.bfloat16
    p.f32 = dt.float32
    p.f32r = dt.float32r

    assert D % 512 == 0 and T % 512 == 0 and S % 1024 == 0
    p.nb, p.T, p.S, p.D = nb, T, S, D
    p.TA = 512    # phase-A token chunk
    p.TBLK = 256  # phase-B/C/D token chunk
    p.KD = D // 128
    p.KE = D // 128
    p.KS = S // 128
    p.NCH = T // p.TBLK
    p.NTT = p.TBLK // 128
    p.SH = S // 2
    p.NSB = p.SH // 512
    p.NEH = D // 512

    nc = bacc.Bacc("TRN2", target_bir_lowering=False, debug=False)
    p.nc = nc

    def din(name, shape, dtype):
        return nc.dram_tensor(name, shape, dtype, kind="ExternalInput").ap()

    p.xt_hi = din("xt_hi", [nb, D, T], p.bf)
    p.xt_lo = din("xt_lo", [nb, D, T], p.bf)
    p.xt_f32 = din("xt_f32", [nb, D, T], p.f32r)
    p.mt_hi = din("mt_hi", [nb, D, S], p.bf)
    p.mt_lo = din("mt_lo", [nb, D, S], p.bf)
    p.mem_n = din("mem_n", [nb, S, D], p.f32r)
    p.w_in_hi = din("w_in_hi", [D, D], p.bf)
    p.w_in_lo = din("w_in_lo", [D, D], p.bf)
    p.w_out_t = din("w_out_t", [2 * D, D], p.f32r)
    p.b_in_rows = din("b_in_rows", [2, D], p.bf)
    p.b_out_row = din("b_out_row", [1, D], p.f32r)
    p.ident_d = din("ident_d", [128, 128], p.f32r)
    p.ones_d = din("ones_d", [1, 128], p.f32r)
    p.h_scr_hi = nc.dram_tensor("h_scr_hi", [nb, D, T], p.bf).ap()
    p.h_scr_lo = nc.dram_tensor("h_scr_lo", [nb, D, T], p.bf).ap()
    p.attn_out = nc.dram_tensor("attn_out", [nb, T, D], p.f32,
                                kind="ExternalOutput").ap()
    p.align_out = nc.dram_tensor("align_out", [nb, T, S], p.f32,
                                 kind="ExternalOutput").ap()

    with tile.TileContext(nc) as tc, ExitStack() as ctx:
        pool = lambda name, bufs, **kw: ctx.enter_context(
            tc.tile_pool(name=name, bufs=bufs, **kw))
        p.consts = pool("consts", 1)
        p.mtp = pool("mt", 1)
        p.statp = pool("stats", 4)
        _build_body(p, tc)

    nc.compile()
    return nc


def _split_bf16(x):
    import ml_dtypes

    bf16 = ml_dtypes.bfloat16
    hi = x.astype(bf16)
    lo = (x - hi.astype(np.float32)).astype(bf16)
    return hi, lo


def kernel(input, memory_bank, W_in, b_in, W_out, b_out):
    from concourse.bass_utils import run_bass_kernel_spmd

    global LAST_EXEC_NS, LAST_RESULTS
    input = np.asarray(input, dtype=np.float32)
    memory_bank = np.asarray(memory_bank, dtype=np.float32)
    W_in = np.asarray(W_in, dtype=np.float32)
    b_in = np.asarray(b_in, dtype=np.float32)
    W_out = np.asarray(W_out, dtype=np.float32)
    b_out = np.asarray(b_out, dtype=np.float32)

    B, T, D = input.shape
    S = memory_bank.shape[1]
    assert B % N_CORES == 0
    nb = B // N_CORES

    key = (nb, T, S, D)
    if key not in _COMPILED:
        _COMPILED[key] = _build(*key)
    nc = _COMPILED[key]

    xt = np.ascontiguousarray(input.transpose(0, 2, 1))  # [B, D, T]
    xt_hi, xt_lo = _split_bf16(xt)
    mt = np.ascontiguousarray(memory_bank.transpose(0, 2, 1))  # [B, D, S]
    mt_hi, mt_lo = _split_bf16(mt)
    w_in_t = np.ascontiguousarray(W_in.T)  # [d, e]
    w_in_hi, w_in_lo = _split_bf16(w_in_t)
    w_out_t = np.ascontiguousarray(W_out.T)  # [f, e] f32
    b_in_rows = np.stack(_split_bf16(b_in))  # [2, D]

    in_maps = []
    for c in range(N_CORES):
        bsl = slice(c * nb, (c + 1) * nb)
        in_maps.append({
            "xt_hi": xt_hi[bsl], "xt_lo": xt_lo[bsl], "xt_f32": xt[bsl],
            "mt_hi": mt_hi[bsl], "mt_lo": mt_lo[bsl],
            "mem_n": memory_bank[bsl],
            "w_in_hi": w_in_hi, "w_in_lo": w_in_lo, "w_out_t": w_out_t,
            "b_in_rows": b_in_rows, "b_out_row": b_out[None, :],
            "ident_d": np.eye(128, dtype=np.float32),
            "ones_d": np.ones((1, 128), dtype=np.float32),
        })

    if PROFILE:
        _install_ntff_shim()
    res = run_bass_kernel_spmd(nc, in_maps, list(range(N_CORES)), trace=PROFILE)
    LAST_RESULTS = res
    if PROFILE:
        LAST_EXEC_NS = res.exec_time_ns
        print(f"HW exec time: {res.exec_time_ns} ns")

    attn_h = np.concatenate([res.results[c]["attn_out"] for c in range(N_CORES)])
    align = np.concatenate([res.results[c]["align_out"] for c in range(N_CORES)])
    return attn_h, align
